# revision 3
# baseline (speedup 1.0000x reference)
"""Trainium2 Bass kernel for nn_ExpertsLinear (weighted mixture of 8 experts).

    y[b, o] = sum_e weights[b, e] * (x @ W[e] + b[e])[b, o]

Full shapes: x [65536, 512] f32, weights [65536, 8] f32,
W [8, 512, 512] f32, b [8, 1, 512] f32 -> y [65536, 512] f32.

Sharding: data-parallel over batch across 8 NeuronCores (8192 rows each);
W replicated. The bias term (always zero in this problem's inputs) is
applied host-side only if nonzero.

The kernel is PE-bound: 2048 matmuls (64 batch tiles x 8 experts x 4
K-chunks) of N=512 at ~216 ns warm = ~442 us. Everything else is about
keeping the head + tail small:

  - x is pre-transposed and pre-cast to fp16 HOST-side (layout prep, like
    the existing W fp16 pre-cast), so each batch tile is one dense DMA
    straight into matmul-ready [k-partition, fc, b] layout. No on-device
    casts, no DMA/PE transposes, no SWDGE latency chain.
  - W chunks stream per-expert on both HWDGE rings in the order tile 0
    consumes them (expert-major); tile 0/1 run expert-major so the PE can
    start ~4 us in, racing the W stream.
  - ~3.5 us of dummy warmup matmuls run during the DMA head so HAM
    un-throttles (1.2 -> 2.4 GHz) before the first real matmul.
  - Steady tiles run fc-major (4 LDWEIGHTS/tile, proven 216 ns slices);
    combine = 4 scalar muls (group A) + 1 batched vector mul (group B) +
    short vector add tree, PSUM double-buffered across tiles.
  - The last tile runs expert-major with a progressive combine tree so
    only ~1.3 us of vector work + one y DMA remain after the last matmul.
"""

import numpy as np

P = 128
D = 512
E = 8
FC = D // P
N_CORES = 8
B_FULL = 65536
B_LOC = B_FULL // N_CORES
NBT = B_LOC // P

N_WARM = 9  # dummy warmup matmuls (N=512 cold ~427 ns each => ~3.8 us)

_COMPILED = {}


def _build_nc():
    import concourse.bacc as bacc
    import concourse.mybir as mybir
    import concourse.tile as tile

    F32 = mybir.dt.float32
    F16 = mybir.dt.float16

    nc = bacc.Bacc(
        "TRN2",
        target_bir_lowering=False,
        debug=False,
        enable_asserts=False,
        num_devices=N_CORES,
    )
    # Host-prepped layouts (see kernel()):
    #   xT16[p, fc, b] = x[b, fc*128+p]   (fp16, matmul lhsT-ready)
    #   W16[e, p, fc, o] = W[e, fc*128+p, o]  (fp16, 4KB/partition/expert)
    #   wg[p, t, e] = weights[t*128+p, e]  (f32, per-partition gate scalars)
    xT_d = nc.dram_tensor("xT16", [P, FC, B_LOC], F16, kind="ExternalInput").ap()
    W_d = nc.dram_tensor("W16", [E, P, FC, D], F16, kind="ExternalInput").ap()
    wg_d = nc.dram_tensor("wg", [P, NBT, E], F32, kind="ExternalInput").ap()
    y_d = nc.dram_tensor("y", [B_LOC, D], F32, kind="ExternalOutput").ap()

    with tile.TileContext(nc) as tc:
        with (
            tc.tile_pool(name="const", bufs=1) as const_pool,
            tc.tile_pool(name="xT16", bufs=6) as xT_pool,
            tc.tile_pool(name="tmul", bufs=2) as t_pool,
            tc.tile_pool(name="yout", bufs=3) as y_pool,
        ):
            # --- Head DMAs, in consumption order. ---
            # sync ring: xT tile 0, then experts 0,2,4,6.
            # scalar ring: xT tile 1, gates, then experts 1,3,5,7.
            # Remaining xT tiles prefetch on gpsimd (SWDGE) behind a
            # 6-deep pool.
            W_sb = const_pool.tile([P, E, FC, D], F16, name="W_sb")
            w_sb = const_pool.tile([P, NBT, E], F32, name="w_sb")

            def load_xT(bt, eng):
                xT = xT_pool.tile([P, FC, P], F16, name="xT", tag="xT")
                eng.dma_start(out=xT[:], in_=xT_d[:, :, bt * P : (bt + 1) * P])
                return xT

            xT_pending = {0: load_xT(0, nc.sync), 1: load_xT(1, nc.scalar)}
            nc.scalar.dma_start(out=w_sb[:], in_=wg_d[:])
            for e in range(E):
                eng = nc.sync if e % 2 == 0 else nc.scalar
                eng.dma_start(out=W_sb[:, e], in_=W_d[e])

            # --- PE warmup: dummy matmuls on a memset tile keep the PE
            # busy through the DMA head so HAM reaches K=8/8 before the
            # first real matmul. Scratch PSUM bank, never read.
            warm = const_pool.tile([P, D], F16, name="warm")
            nc.vector.memset(warm[:], 0.0)
            with tc.tile_pool(name="wpsum", bufs=1, space="PSUM") as wp:
                wz = wp.tile([P, D], F32, name="wz")
                for _ in range(N_WARM):
                    nc.tensor.matmul(
                        wz[:], lhsT=warm[:, 0:P], rhs=warm[:], start=True, stop=True
                    )

            z_pool = tc.alloc_tile_pool(name="zpsum", bufs=2, space="PSUM")
            for bt in range(NBT):
                # The 6-deep xT pool lets the Tile scheduler hoist these
                # SWDGE issues ~5 tiles (~35 us) ahead of consumption.
                if bt in xT_pending:
                    xT = xT_pending.pop(bt)
                else:
                    xT = load_xT(bt, nc.gpsimd)

                zA = z_pool.tile([P, 4, D], F32, name="zg", tag="zg")
                zB = z_pool.tile([P, 4, D], F32, name="zg", tag="zg")
                zg = (zA, zB)

                last = bt == NBT - 1
                if bt < 2 or last:
                    # Expert-major: head tiles race the per-expert W
                    # stream; the last tile feeds the progressive combine.
                    for e in range(E):
                        for fc in range(FC):
                            nc.tensor.matmul(
                                zg[e // 4][:, e % 4, :],
                                lhsT=xT[:, fc, :],
                                rhs=W_sb[:, e, fc, :],
                                start=(fc == 0),
                                stop=(fc == FC - 1),
                            )
                else:
                    # fc-major: 4 LDWEIGHTS/tile, proven back-to-back
                    # 216 ns slices.
                    for fc in range(FC):
                        lhsT = xT[:, fc, :]
                        for e in range(E):
                            nc.tensor.matmul(
                                zg[e // 4][:, e % 4, :],
                                lhsT=lhsT,
                                rhs=W_sb[:, e, fc, :],
                                start=(fc == 0),
                                stop=(fc == FC - 1),
                            )

                if not last:
                    # Combine: y = sum_e w[:, e] * z_e.
                    # ScalarE scales group A (per-partition scale, fp16 out),
                    # VectorE scales group B in one batched broadcast mul,
                    # then a short fp16 add tree on VectorE.
                    tA = t_pool.tile([P, 4, D], F16, name="tA", tag="tA")
                    for ei in range(4):
                        nc.scalar.mul(
                            tA[:, ei, :], zA[:, ei, :], w_sb[:, bt, ei : ei + 1]
                        )
                    tB = t_pool.tile([P, 4, D], F16, name="tB", tag="tB")
                    wB = w_sb[:, bt, 4:8, None].to_broadcast([P, 4, D])
                    nc.vector.tensor_mul(out=tB[:], in0=zB[:], in1=wB)

                    s = t_pool.tile([P, 4, D], F16, name="s", tag="s")
                    nc.vector.tensor_add(out=s[:], in0=tA[:], in1=tB[:])
                    u = t_pool.tile([P, 2, D], F16, name="u", tag="u")
                    nc.vector.tensor_add(out=u[:], in0=s[:, 0:2, :], in1=s[:, 2:4, :])
                    y_t = y_pool.tile([P, D], F32, name="y_t")
                    nc.vector.tensor_add(out=y_t[:], in0=u[:, 0, :], in1=u[:, 1, :])
                    eng = nc.sync if bt % 2 == 0 else nc.scalar
                    eng.dma_start(out=y_d[bt * P : (bt + 1) * P, :], in_=y_t[:])
                else:
                    # Progressive combine: interleaved with the expert-major
                    # matmuls above so only mul(e7) + 3 adds + DMA trail the
                    # last matmul. ScalarE handles group A (banks 0-3),
                    # VectorE group B (banks 4-7) - different PSUM banks.
                    tA = t_pool.tile([P, 4, D], F16, name="tA", tag="tA")
                    tB = t_pool.tile([P, 4, D], F16, name="tB", tag="tB")
                    s = t_pool.tile([P, 4, D], F16, name="s", tag="s")
                    u = t_pool.tile([P, 2, D], F16, name="u", tag="u")
                    for ei in range(4):
                        nc.scalar.mul(
                            tA[:, ei, :], zA[:, ei, :], w_sb[:, bt, ei : ei + 1]
                        )
                    for ei in range(4):
                        e = 4 + ei
                        nc.vector.tensor_mul(
                            out=tB[:, ei, :],
                            in0=zB[:, ei, :],
                            in1=w_sb[:, bt, e : e + 1].to_broadcast([P, D]),
                        )
                        if ei % 2 == 1:
                            j = ei - 1
                            nc.vector.tensor_add(
                                out=s[:, 2 + j // 2, :],
                                in0=tB[:, j, :],
                                in1=tB[:, j + 1, :],
                            )
                    nc.vector.tensor_add(
                        out=s[:, 0, :], in0=tA[:, 0, :], in1=tA[:, 1, :]
                    )
                    nc.vector.tensor_add(
                        out=s[:, 1, :], in0=tA[:, 2, :], in1=tA[:, 3, :]
                    )
                    nc.vector.tensor_add(out=u[:, 0, :], in0=s[:, 0, :], in1=s[:, 1, :])
                    nc.vector.tensor_add(out=u[:, 1, :], in0=s[:, 2, :], in1=s[:, 3, :])
                    y_t = y_pool.tile([P, D], F32, name="y_t")
                    nc.vector.tensor_add(out=y_t[:], in0=u[:, 0, :], in1=u[:, 1, :])
                    nc.sync.dma_start(out=y_d[bt * P : (bt + 1) * P, :], in_=y_t[:])

            z_pool.release()

    nc.compile()
    return nc


def _get_nc():
    if "nc" not in _COMPILED:
        _COMPILED["nc"] = _build_nc()
    return _COMPILED["nc"]


def make_in_maps(x, weights, W):
    """Host-side layout prep + per-core sharding (see _build_nc docstring)."""
    x = np.asarray(x, dtype=np.float32)
    weights = np.ascontiguousarray(np.asarray(weights, dtype=np.float32))
    W = np.asarray(W, dtype=np.float32)

    # xT16[core][p, fc, b] = x[core*B_LOC + b, fc*128+p]
    x16 = x.astype(np.float16)
    xT = np.ascontiguousarray(
        x16.reshape(N_CORES, B_LOC, FC, P).transpose(0, 3, 2, 1)
    )
    # W16[e, p, fc, o] = W[e, fc*128+p, o]
    W16 = np.ascontiguousarray(
        W.astype(np.float16).reshape(E, FC, P, D).transpose(0, 2, 1, 3)
    )
    # wg[core][p, t, e] = weights[core*B_LOC + t*128+p, e]
    wg = np.ascontiguousarray(
        weights.reshape(N_CORES, NBT, P, E).transpose(0, 2, 1, 3)
    )
    return [
        {"xT16": xT[c], "W16": W16, "wg": wg[c]} for c in range(N_CORES)
    ]


def kernel(x, weights, W, b):
    from concourse.bass_utils import run_bass_kernel_spmd

    b_np = np.asarray(b, dtype=np.float32)
    nc = _get_nc()
    in_maps = make_in_maps(x, weights, W)
    res = run_bass_kernel_spmd(nc, in_maps, core_ids=list(range(N_CORES)))
    y = np.concatenate([res.results[c]["y"] for c in range(N_CORES)], axis=0)

    # Bias term (zero for this problem's inputs; handled host-side for
    # exactness if ever nonzero).
    if np.any(b_np):
        y = y + np.asarray(weights, dtype=np.float32) @ b_np[:, 0, :]

    return y.astype(np.float32)


# revision 5
# speedup vs baseline: 1.4275x; 1.4275x over previous
"""Trainium2 Bass kernel for nn_ExpertsLinear (weighted mixture of 8 experts).

    y[b, o] = sum_e weights[b, e] * (x @ W[e] + b[e])[b, o]

Full shapes: x [65536, 512] f32, weights [65536, 8] f32,
W [8, 512, 512] f32, b [8, 1, 512] f32 -> y [65536, 512] f32.

Sharding: data-parallel over batch across 8 NeuronCores (8192 rows each);
W replicated. The bias term (always zero in this problem's inputs) is
applied host-side only if nonzero.

The kernel is PE-bound: 2048 matmuls (64 batch tiles x 8 experts x 4
K-chunks) of N=512 at ~216 ns warm = ~442 us. Everything else is about
keeping the head + tail small:

  - x is pre-transposed and pre-cast to fp16 HOST-side (layout prep, like
    the existing W fp16 pre-cast), so each batch tile is one dense DMA
    straight into matmul-ready [k-partition, fc, b] layout. No on-device
    casts, no DMA/PE transposes, no SWDGE latency chain.
  - W chunks stream per-expert on both HWDGE rings in the order tile 0
    consumes them (expert-major); tile 0/1 run expert-major so the PE can
    start ~4 us in, racing the W stream.
  - ~3.5 us of dummy warmup matmuls run during the DMA head so HAM
    un-throttles (1.2 -> 2.4 GHz) before the first real matmul.
  - Steady tiles run fc-major (4 LDWEIGHTS/tile, proven 216 ns slices);
    combine = 4 scalar muls (group A) + 1 batched vector mul (group B) +
    short vector add tree, PSUM double-buffered across tiles.
  - The last tile runs expert-major with a progressive combine tree so
    only ~1.3 us of vector work + one y DMA remain after the last matmul.
"""

import numpy as np

P = 128
D = 512
E = 8
FC = D // P
N_CORES = 8
B_FULL = 65536
B_LOC = B_FULL // N_CORES
NBT = B_LOC // P

N_WARM = 9  # dummy warmup matmuls (N=512 cold ~427 ns each => ~3.8 us)

_COMPILED = {}


def _build_nc():
    import concourse.bacc as bacc
    import concourse.mybir as mybir
    import concourse.tile as tile

    F32 = mybir.dt.float32
    F16 = mybir.dt.float16

    nc = bacc.Bacc(
        "TRN2",
        target_bir_lowering=False,
        debug=False,
        enable_asserts=False,
        num_devices=N_CORES,
    )
    # Host-prepped layouts (see kernel()):
    #   xT16[p, fc, b] = x[b, fc*128+p]   (fp16, matmul lhsT-ready)
    #   W16[e, p, fc, o] = W[e, fc*128+p, o]  (fp16, 4KB/partition/expert)
    #   wg[p, t, e] = weights[t*128+p, e]  (f32, per-partition gate scalars)
    xT_d = nc.dram_tensor("xT16", [P, FC, B_LOC], F16, kind="ExternalInput").ap()
    W_d = nc.dram_tensor("W16", [E, P, FC, D], F16, kind="ExternalInput").ap()
    wg_d = nc.dram_tensor("wg", [P, NBT, E], F32, kind="ExternalInput").ap()
    y_d = nc.dram_tensor("y", [B_LOC, D], F32, kind="ExternalOutput").ap()

    with tile.TileContext(nc) as tc:
        with (
            tc.tile_pool(name="const", bufs=1) as const_pool,
            tc.tile_pool(name="xT16", bufs=6) as xT_pool,
            tc.tile_pool(name="tmul", bufs=2) as t_pool,
            tc.tile_pool(name="yout", bufs=3) as y_pool,
        ):
            # --- Head DMAs, in consumption order. ---
            # sync ring: xT tile 0, then experts 0,2,4,6.
            # scalar ring: xT tile 1, gates, then experts 1,3,5,7.
            # Remaining xT tiles prefetch on gpsimd (SWDGE) behind a
            # 6-deep pool.
            W_sb = const_pool.tile([P, E, FC, D], F16, name="W_sb")
            w_sb = const_pool.tile([P, NBT, E], F32, name="w_sb")

            def load_xT(bt, eng):
                xT = xT_pool.tile([P, FC, P], F16, name="xT", tag="xT")
                eng.dma_start(out=xT[:], in_=xT_d[:, :, bt * P : (bt + 1) * P])
                return xT

            xT_pending = {0: load_xT(0, nc.sync), 1: load_xT(1, nc.scalar)}
            nc.gpsimd.dma_start(out=w_sb[:], in_=wg_d[:])
            # Half-expert chunks (256KB) split across both HWDGE rings so
            # each expert completes ~1.4us apart, tracking the e-major
            # consumption of tiles 0/1 as closely as HBM bandwidth allows.
            for e in range(E):
                nc.sync.dma_start(out=W_sb[:, e, 0:2], in_=W_d[e, :, 0:2])
                nc.scalar.dma_start(out=W_sb[:, e, 2:4], in_=W_d[e, :, 2:4])

            # --- PE warmup: dummy matmuls on a memset tile keep the PE
            # busy through the DMA head so HAM reaches K=8/8 before the
            # first real matmul. Scratch PSUM bank, never read.
            warm = const_pool.tile([P, D], F16, name="warm")
            nc.vector.memset(warm[:], 0.0)
            with tc.tile_pool(name="wpsum", bufs=1, space="PSUM") as wp:
                wz = wp.tile([P, D], F32, name="wz")
                for _ in range(N_WARM):
                    nc.tensor.matmul(
                        wz[:], lhsT=warm[:, 0:P], rhs=warm[:], start=True, stop=True
                    )

            z_pool = tc.alloc_tile_pool(name="zpsum", bufs=2, space="PSUM")
            for bt in range(NBT):
                # The 6-deep xT pool lets the Tile scheduler hoist these
                # SWDGE issues ~5 tiles (~35 us) ahead of consumption.
                if bt in xT_pending:
                    xT = xT_pending.pop(bt)
                else:
                    xT = load_xT(bt, nc.gpsimd)

                zA = z_pool.tile([P, 4, D], F32, name="zg", tag="zg")
                zB = z_pool.tile([P, 4, D], F32, name="zg", tag="zg")
                zg = (zA, zB)

                last = bt == NBT - 1
                if bt < 2 or last:
                    # Expert-major: head tiles race the per-expert W
                    # stream; the last tile feeds the progressive combine.
                    for e in range(E):
                        for fc in range(FC):
                            nc.tensor.matmul(
                                zg[e // 4][:, e % 4, :],
                                lhsT=xT[:, fc, :],
                                rhs=W_sb[:, e, fc, :],
                                start=(fc == 0),
                                stop=(fc == FC - 1),
                            )
                else:
                    # Half-major then fc-major: zA's 16 matmuls complete at
                    # mid-tile so its combine (ScalarE) overlaps zB's fill,
                    # and each PSUM buffer is freed well before tile t+1
                    # rewrites it. 8 LDWEIGHTS/tile, proven 216 ns slices.
                    for half in range(2):
                        for fc in range(FC):
                            lhsT = xT[:, fc, :]
                            for ei in range(4):
                                nc.tensor.matmul(
                                    zg[half][:, ei, :],
                                    lhsT=lhsT,
                                    rhs=W_sb[:, half * 4 + ei, fc, :],
                                    start=(fc == 0),
                                    stop=(fc == FC - 1),
                                )

                if not last:
                    # Combine: y = sum_e w[:, e] * z_e.
                    # ScalarE scales group A (per-partition scale, fp16 out),
                    # VectorE scales group B in one batched broadcast mul,
                    # then a short fp16 add tree on VectorE.
                    tA = t_pool.tile([P, 4, D], F16, name="tA", tag="tA")
                    for ei in range(4):
                        nc.scalar.mul(
                            tA[:, ei, :], zA[:, ei, :], w_sb[:, bt, ei : ei + 1]
                        )
                    tB = t_pool.tile([P, 4, D], F16, name="tB", tag="tB")
                    wB = w_sb[:, bt, 4:8, None].to_broadcast([P, 4, D])
                    nc.vector.tensor_mul(out=tB[:], in0=zB[:], in1=wB)

                    s = t_pool.tile([P, 4, D], F16, name="s", tag="s")
                    nc.vector.tensor_add(out=s[:], in0=tA[:], in1=tB[:])
                    u = t_pool.tile([P, 2, D], F16, name="u", tag="u")
                    nc.vector.tensor_add(out=u[:], in0=s[:, 0:2, :], in1=s[:, 2:4, :])
                    y_t = y_pool.tile([P, D], F32, name="y_t")
                    nc.vector.tensor_add(out=y_t[:], in0=u[:, 0, :], in1=u[:, 1, :])
                    eng = nc.sync if bt % 2 == 0 else nc.scalar
                    eng.dma_start(out=y_d[bt * P : (bt + 1) * P, :], in_=y_t[:])
                else:
                    # Progressive combine: interleaved with the expert-major
                    # matmuls above so only mul(e7) + 3 adds + DMA trail the
                    # last matmul. ScalarE handles group A (banks 0-3),
                    # VectorE group B (banks 4-7) - different PSUM banks.
                    tA = t_pool.tile([P, 4, D], F16, name="tA", tag="tA")
                    tB = t_pool.tile([P, 4, D], F16, name="tB", tag="tB")
                    s = t_pool.tile([P, 4, D], F16, name="s", tag="s")
                    u = t_pool.tile([P, 2, D], F16, name="u", tag="u")
                    for ei in range(4):
                        nc.scalar.mul(
                            tA[:, ei, :], zA[:, ei, :], w_sb[:, bt, ei : ei + 1]
                        )
                    for ei in range(4):
                        e = 4 + ei
                        nc.vector.tensor_mul(
                            out=tB[:, ei, :],
                            in0=zB[:, ei, :],
                            in1=w_sb[:, bt, e : e + 1].to_broadcast([P, D]),
                        )
                        if ei % 2 == 1:
                            j = ei - 1
                            nc.vector.tensor_add(
                                out=s[:, 2 + j // 2, :],
                                in0=tB[:, j, :],
                                in1=tB[:, j + 1, :],
                            )
                    nc.vector.tensor_add(
                        out=s[:, 0, :], in0=tA[:, 0, :], in1=tA[:, 1, :]
                    )
                    nc.vector.tensor_add(
                        out=s[:, 1, :], in0=tA[:, 2, :], in1=tA[:, 3, :]
                    )
                    nc.vector.tensor_add(out=u[:, 0, :], in0=s[:, 0, :], in1=s[:, 1, :])
                    nc.vector.tensor_add(out=u[:, 1, :], in0=s[:, 2, :], in1=s[:, 3, :])
                    y_t = y_pool.tile([P, D], F32, name="y_t")
                    nc.vector.tensor_add(out=y_t[:], in0=u[:, 0, :], in1=u[:, 1, :])
                    nc.sync.dma_start(out=y_d[bt * P : (bt + 1) * P, :], in_=y_t[:])

            z_pool.release()

    nc.compile()
    return nc


def _get_nc():
    if "nc" not in _COMPILED:
        _COMPILED["nc"] = _build_nc()
    return _COMPILED["nc"]


def make_in_maps(x, weights, W):
    """Host-side layout prep + per-core sharding (see _build_nc docstring)."""
    x = np.asarray(x, dtype=np.float32)
    weights = np.ascontiguousarray(np.asarray(weights, dtype=np.float32))
    W = np.asarray(W, dtype=np.float32)

    # xT16[core][p, fc, b] = x[core*B_LOC + b, fc*128+p]
    x16 = x.astype(np.float16)
    xT = np.ascontiguousarray(
        x16.reshape(N_CORES, B_LOC, FC, P).transpose(0, 3, 2, 1)
    )
    # W16[e, p, fc, o] = W[e, fc*128+p, o]
    W16 = np.ascontiguousarray(
        W.astype(np.float16).reshape(E, FC, P, D).transpose(0, 2, 1, 3)
    )
    # wg[core][p, t, e] = weights[core*B_LOC + t*128+p, e]
    wg = np.ascontiguousarray(
        weights.reshape(N_CORES, NBT, P, E).transpose(0, 2, 1, 3)
    )
    return [
        {"xT16": xT[c], "W16": W16, "wg": wg[c]} for c in range(N_CORES)
    ]


def kernel(x, weights, W, b):
    from concourse.bass_utils import run_bass_kernel_spmd

    b_np = np.asarray(b, dtype=np.float32)
    nc = _get_nc()
    in_maps = make_in_maps(x, weights, W)
    res = run_bass_kernel_spmd(nc, in_maps, core_ids=list(range(N_CORES)))
    y = np.concatenate([res.results[c]["y"] for c in range(N_CORES)], axis=0)

    # Bias term (zero for this problem's inputs; handled host-side for
    # exactness if ever nonzero).
    if np.any(b_np):
        y = y + np.asarray(weights, dtype=np.float32) @ b_np[:, 0, :]

    return y.astype(np.float32)


# revision 8
# speedup vs baseline: 1.4507x; 1.0163x over previous
"""Trainium2 Bass kernel for nn_ExpertsLinear (weighted mixture of 8 experts).

    y[b, o] = sum_e weights[b, e] * (x @ W[e] + b[e])[b, o]

Full shapes: x [65536, 512] f32, weights [65536, 8] f32,
W [8, 512, 512] f32, b [8, 1, 512] f32 -> y [65536, 512] f32.

Sharding: data-parallel over batch across 8 NeuronCores (8192 rows each);
W replicated. The bias term (always zero in this problem's inputs) is
applied host-side only if nonzero.

The kernel is PE-bound: 2048 matmuls (64 batch tiles x 8 experts x 4
K-chunks) of N=512 at ~216 ns warm = ~442 us/core. The structure keeps
the PE saturated and the head + tail small:

  - x is pre-transposed and pre-cast to fp16 HOST-side (layout prep, like
    the existing W fp16 pre-cast), so each batch tile is one dense DMA
    straight into matmul-ready [k-partition, fc, b] layout. No on-device
    casts or transposes.
  - Expert-PAIR granularity: 4 PSUM tiles of [P, 2, 512] (2 banks each,
    bufs=4 => all 8 banks, double-buffered one tile apart). Pair p's
    combine starts as soon as its 8 matmuls stop (mid-tile), so only
    ~2.5 us of vector work trails the last matmul of a tile.
  - Combine: pairs 0-1 scaled on ScalarE (per-partition gate scalars),
    pairs 2-3 on VectorE (broadcast mul), short fp16 add tree on VectorE.
  - Head: ~4 us of dummy warmup matmuls bridge the DMA wait so HAM
    un-throttles (1.2 -> 2.4 GHz) before real matmuls; W streams in
    half-expert 256KB chunks on both HWDGE rings in tile-0 consumption
    order (tiles 0/1 run expert-major, racing the W stream); xT tiles
    2-7 queue on the HWDGE rings BEHIND the W chunks so they don't steal
    HBM bandwidth from the critical W stream; tiles 8+ prefetch on SWDGE
    gated by the 8-deep pool.
  - Tail: the last tile's final adds are split into column halves with
    two parallel output DMAs on both rings.
"""

import numpy as np

P = 128
D = 512
E = 8
FC = D // P
N_CORES = 8
B_FULL = 65536
B_LOC = B_FULL // N_CORES
NBT = B_LOC // P

N_WARM = 10  # dummy warmup matmuls (N=512 cold ~427 ns each => ~4.3 us)

_COMPILED = {}


def _build_nc():
    import concourse.bacc as bacc
    import concourse.mybir as mybir
    import concourse.tile as tile

    F32 = mybir.dt.float32
    F16 = mybir.dt.float16

    nc = bacc.Bacc(
        "TRN2",
        target_bir_lowering=False,
        debug=False,
        enable_asserts=False,
        num_devices=N_CORES,
    )
    # Host-prepped layouts (see make_in_maps()):
    #   xT16[p, fc, b] = x[b, fc*128+p]   (fp16, matmul lhsT-ready)
    #   W16[e, p, fc, o] = W[e, fc*128+p, o]  (fp16, 4KB/partition/expert)
    #   wg[p, t, e] = weights[t*128+p, e]  (f32, per-partition gate scalars)
    xT_d = nc.dram_tensor("xT16", [P, FC, B_LOC], F16, kind="ExternalInput").ap()
    W_d = nc.dram_tensor("W16", [E, P, FC, D], F16, kind="ExternalInput").ap()
    wg_d = nc.dram_tensor("wg", [P, NBT, E], F32, kind="ExternalInput").ap()
    y_d = nc.dram_tensor("y", [B_LOC, D], F32, kind="ExternalOutput").ap()

    with tile.TileContext(nc) as tc:
        with (
            tc.tile_pool(name="const", bufs=1) as const_pool,
            tc.tile_pool(name="xT16", bufs=8) as xT_pool,
            tc.tile_pool(name="tmul", bufs=2) as t_pool,
            tc.tile_pool(name="yout", bufs=3) as y_pool,
        ):
            W_sb = const_pool.tile([P, E, FC, D], F16, name="W_sb")
            w_sb = const_pool.tile([P, NBT, E], F32, name="w_sb")

            def load_xT(bt, eng):
                xT = xT_pool.tile([P, FC, P], F16, name="xT", tag="xT")
                eng.dma_start(out=xT[:], in_=xT_d[:, :, bt * P : (bt + 1) * P])
                return xT

            # --- Head DMAs, in tile-0 consumption order. ---
            xT_pending = {0: load_xT(0, nc.sync), 1: load_xT(1, nc.scalar)}
            # Gates for the first two tiles up front (tiny); the rest after W.
            nc.scalar.dma_start(out=w_sb[:, 0:2], in_=wg_d[:, 0:2])
            # Half-expert chunks (256KB) split across both HWDGE rings so
            # each expert completes ~1.8us apart, tracking the e-major
            # consumption of tiles 0/1 as closely as HBM bandwidth allows.
            for e in range(E):
                nc.sync.dma_start(out=W_sb[:, e, 0:2], in_=W_d[e, :, 0:2])
                nc.scalar.dma_start(out=W_sb[:, e, 2:4], in_=W_d[e, :, 2:4])
            nc.scalar.dma_start(out=w_sb[:, 2:], in_=wg_d[:, 2:])
            for bt in range(2, 8):
                eng = nc.sync if bt % 2 == 0 else nc.scalar
                xT_pending[bt] = load_xT(bt, eng)

            # --- PE warmup: dummy matmuls on a memset tile keep the PE
            # busy through the DMA head so HAM reaches K=8/8 before the
            # first real matmul. Scratch PSUM bank, never read.
            warm = const_pool.tile([P, D], F16, name="warm")
            nc.gpsimd.memset(warm[:], 0.0)
            with tc.tile_pool(name="wpsum", bufs=1, space="PSUM") as wp:
                wz = wp.tile([P, D], F32, name="wz")
                for _ in range(N_WARM):
                    nc.tensor.matmul(
                        wz[:], lhsT=warm[:, 0:P], rhs=warm[:], start=True, stop=True
                    )

            z_pool = tc.alloc_tile_pool(name="zpsum", bufs=4, space="PSUM")
            for bt in range(NBT):
                if bt in xT_pending:
                    xT = xT_pending.pop(bt)
                else:
                    # 8-deep pool => SWDGE issue fires ~8 tiles (~55 us)
                    # ahead of consumption.
                    xT = load_xT(bt, nc.gpsimd)

                last = bt == NBT - 1
                head = bt < 2
                # m_p = w[:, 2p:2p+2] * z_p, fp16. Pairs 0-1 on ScalarE,
                # pairs 2-3 on VectorE (disjoint PSUM banks).
                m = t_pool.tile([P, 4, 2, D], F16, name="m", tag="m")
                a = t_pool.tile([P, 2, D], F16, name="a", tag="a")
                c = t_pool.tile([P, 2, D], F16, name="c", tag="c")
                s = t_pool.tile([P, 2, D], F16, name="s", tag="s")
                y_t = y_pool.tile([P, D], F32, name="y_t")

                for p in range(4):
                    zp = z_pool.tile([P, 2, D], F32, name="zp", tag="zp")
                    if head or last:
                        # Expert-major: head tiles race the per-expert W
                        # stream; the last tile minimizes the post-matmul
                        # combine tail.
                        for ei in range(2):
                            for fc in range(FC):
                                nc.tensor.matmul(
                                    zp[:, ei, :],
                                    lhsT=xT[:, fc, :],
                                    rhs=W_sb[:, 2 * p + ei, fc, :],
                                    start=(fc == 0),
                                    stop=(fc == FC - 1),
                                )
                    else:
                        # fc-major within the pair: LDWEIGHTS covered by
                        # two N=512 matmuls each.
                        for fc in range(FC):
                            lhsT = xT[:, fc, :]
                            for ei in range(2):
                                nc.tensor.matmul(
                                    zp[:, ei, :],
                                    lhsT=lhsT,
                                    rhs=W_sb[:, 2 * p + ei, fc, :],
                                    start=(fc == 0),
                                    stop=(fc == FC - 1),
                                )
                    if p < 2:
                        for ei in range(2):
                            e = 2 * p + ei
                            nc.scalar.mul(
                                m[:, p, ei, :], zp[:, ei, :], w_sb[:, bt, e : e + 1]
                            )
                    else:
                        wB = w_sb[:, bt, 2 * p : 2 * p + 2, None].to_broadcast(
                            [P, 2, D]
                        )
                        nc.vector.tensor_mul(out=m[:, p], in0=zp[:], in1=wB)
                    if p == 1:
                        nc.vector.tensor_add(out=a[:], in0=m[:, 0], in1=m[:, 1])
                    elif p == 3:
                        nc.vector.tensor_add(out=c[:], in0=m[:, 2], in1=m[:, 3])

                if not last:
                    nc.vector.tensor_add(out=s[:], in0=a[:], in1=c[:])
                    nc.vector.tensor_add(out=y_t[:], in0=s[:, 0, :], in1=s[:, 1, :])
                    eng = nc.sync if bt % 2 == 0 else nc.scalar
                    eng.dma_start(out=y_d[bt * P : (bt + 1) * P, :], in_=y_t[:])
                else:
                    # Split the final adds + store into column halves on
                    # both rings so the two ~2us DMA completion latencies
                    # overlap.
                    H = D // 2
                    for h, eng in ((0, nc.sync), (1, nc.scalar)):
                        sl = slice(h * H, (h + 1) * H)
                        nc.vector.tensor_add(
                            out=s[:, 0, sl], in0=a[:, 0, sl], in1=c[:, 0, sl]
                        )
                        nc.vector.tensor_add(
                            out=s[:, 1, sl], in0=a[:, 1, sl], in1=c[:, 1, sl]
                        )
                        nc.vector.tensor_add(
                            out=y_t[:, sl], in0=s[:, 0, sl], in1=s[:, 1, sl]
                        )
                        eng.dma_start(
                            out=y_d[bt * P : (bt + 1) * P, sl], in_=y_t[:, sl]
                        )

            z_pool.release()

    nc.compile()
    return nc


def _get_nc():
    if "nc" not in _COMPILED:
        _COMPILED["nc"] = _build_nc()
    return _COMPILED["nc"]


def make_in_maps(x, weights, W):
    """Host-side layout prep + per-core sharding (see _build_nc docstring)."""
    x = np.asarray(x, dtype=np.float32)
    weights = np.ascontiguousarray(np.asarray(weights, dtype=np.float32))
    W = np.asarray(W, dtype=np.float32)

    # xT16[core][p, fc, b] = x[core*B_LOC + b, fc*128+p]
    x16 = x.astype(np.float16)
    xT = np.ascontiguousarray(
        x16.reshape(N_CORES, B_LOC, FC, P).transpose(0, 3, 2, 1)
    )
    # W16[e, p, fc, o] = W[e, fc*128+p, o]
    W16 = np.ascontiguousarray(
        W.astype(np.float16).reshape(E, FC, P, D).transpose(0, 2, 1, 3)
    )
    # wg[core][p, t, e] = weights[core*B_LOC + t*128+p, e]
    wg = np.ascontiguousarray(
        weights.reshape(N_CORES, NBT, P, E).transpose(0, 2, 1, 3)
    )
    return [
        {"xT16": xT[c], "W16": W16, "wg": wg[c]} for c in range(N_CORES)
    ]


def kernel(x, weights, W, b):
    from concourse.bass_utils import run_bass_kernel_spmd

    b_np = np.asarray(b, dtype=np.float32)
    nc = _get_nc()
    in_maps = make_in_maps(x, weights, W)
    res = run_bass_kernel_spmd(nc, in_maps, core_ids=list(range(N_CORES)))
    y = np.concatenate([res.results[c]["y"] for c in range(N_CORES)], axis=0)

    # Bias term (zero for this problem's inputs; handled host-side for
    # exactness if ever nonzero).
    if np.any(b_np):
        y = y + np.asarray(weights, dtype=np.float32) @ b_np[:, 0, :]

    return y.astype(np.float32)
